# revision 1
# baseline (speedup 1.0000x reference)
"""AriaGroupedGEMM (MoE grouped GEMM) on 8 TRN2 NeuronCores.

Problem: input [4096, 2048] f32, weight [8, 2048, 2048] f32,
tokens_per_expert [8] int32 (tokens pre-sorted by expert).
out[i] = input[i] @ weight[expert_of(i)].

Strategy: expert-parallel. Core g owns expert g's weight and its token
group (boundaries computed on host from tokens_per_expert). Each core
runs a dense [T_pad, 2048] @ [2048, 2048] GEMM in bf16 (fp32 PSUM
accumulation). Host pre-swizzles operands into SBUF-native layouts so
every DMA is fully contiguous, and gathers/unpads the result.

Schedule: all input DMAs go on the sync HWDGE ring in exact consumption
order (FIFO drain => just-in-time arrival). xt is chunked per m-tile and
the first two n-blocks' weights are chunked on k so the PE starts after
~1MB and never stalls once streaming. Warm-up matmuls on scratch data
lift the HAM clock gate before the real stream begins.
"""
import sys
import functools

for _p in ("/opt/trn_rl_repo", "/root/.axon_site/_ro/trn_rl_repo"):
    if _p not in sys.path:
        sys.path.insert(0, _p)

import numpy as np
import ml_dtypes

import concourse.mybir as mybir
import concourse.tile as tile
from concourse import bacc
from concourse import bass_utils

P = 128
K = 2048            # in_features (contraction)
N = 2048            # out_features
G = 8               # experts == cores
KO = K // P         # 16 k-subtiles
NB = N // 512       # 4 n-blocks of 512

COMPUTE_DT = mybir.dt.bfloat16
NP_COMPUTE = ml_dtypes.bfloat16
OUT_DT = mybir.dt.bfloat16      # psum(f32) -> bf16 on the way out; host upcasts

N_WARMUP_MM = 9     # N=512 warm-up matmuls (HAM ramp) before data lands
N_FILLER_MM = 0     # gap-filler matmuls inside the DMA-bound head phase


@functools.lru_cache(maxsize=4)
def _build(t_pad: int):
    """Build + compile the per-core GEMM graph for token-pad t_pad."""
    mt = t_pad // P  # m tiles of 128 tokens

    nc = bacc.Bacc("TRN2", target_bir_lowering=False, debug=False)

    # host-swizzled SBUF-native layouts (contiguous per partition line):
    # xt[mi, p, ko, j] = X[mi*P + j, ko*P + p]
    # w[p, ko, j]      = W[ko*P + p, j]
    xt_d = nc.dram_tensor(
        "xt", [mt, P, KO, P], COMPUTE_DT, kind="ExternalInput"
    ).ap()
    w_d = nc.dram_tensor(
        "w", [P, KO, N], COMPUTE_DT, kind="ExternalInput"
    ).ap()
    out_d = nc.dram_tensor("out", [t_pad, N], OUT_DT, kind="ExternalOutput").ap()

    # column blocks: two narrow head blocks shrink the DMA bytes gating the
    # first outputs; the tail blocks run at the efficient 512-wide MM rate
    if N == 2048:
        BLOCKS = [(0, 256), (256, 256), (512, 512), (1024, 512), (1536, 512)]
    else:
        BLOCKS = [(i * 512, 512) for i in range(N // 512)]
    NBK = len(BLOCKS)

    WCH = 4             # ko per w DMA chunk (k-pacing granularity)
    NCH = KO // WCH     # 4 chunks per block

    with tile.TileContext(nc) as tc:
        with (
            tc.tile_pool(name="xt_p", bufs=1) as xt_p,
            tc.tile_pool(name="w_p", bufs=1) as w_p,
            tc.tile_pool(name="o_p", bufs=4) as o_p,
            tc.tile_pool(name="wu_p", bufs=1) as wu_p,
            tc.tile_pool(name="ps", bufs=7, space="PSUM") as ps,
            tc.tile_pool(name="wu_ps_p", bufs=1, space="PSUM") as wu_ps_p,
        ):
            # --- PE warm-up: matmuls on scratch zeros, no DMA deps. They
            # run during the initial DMA wait and lift the HAM clock gate
            # toward 2.4GHz before the real stream starts.
            wu_lhs = wu_p.tile([P, P], COMPUTE_DT, tag="wu_lhs")
            wu_rhs = wu_p.tile([P, 512], COMPUTE_DT, tag="wu_rhs")
            nc.gpsimd.memset(wu_lhs[:], 0.0)
            nc.gpsimd.memset(wu_rhs[:], 0.0)
            wu_ps = wu_ps_p.tile([P, 512], mybir.dt.float32, tag="wu_ps")
            for i in range(N_WARMUP_MM):
                nc.tensor.matmul(wu_ps[:], wu_lhs[:], wu_rhs[:],
                                 start=(i == 0), stop=False,
                                 skip_group_check=True)

            # --- input DMAs, all on sync, in consumption order
            xt_t = [None] * mt
            w_c = [dict() for _ in range(NBK)]  # b -> ko -> (tile, off)

            def load_xt(mi):
                t = xt_p.tile([P, KO, P], COMPUTE_DT, tag=f"xt_m{mi}",
                              name=f"xt_m{mi}")
                nc.sync.dma_start(t[:], xt_d[mi])
                xt_t[mi] = t

            def load_w_chunk(b, c):
                c0, width = BLOCKS[b]
                ko0 = c * WCH
                t = w_p.tile([P, WCH, width], COMPUTE_DT, tag=f"w_b{b}_c{c}",
                             name=f"w_b{b}_c{c}")
                nc.sync.dma_start(t[:], w_d[:, ko0:ko0 + WCH, c0:c0 + width])
                for i in range(WCH):
                    w_c[b][ko0 + i] = (t, i)

            # diagonal head schedule needs one psum bank per m-tile
            diag_b0 = mt <= 6 and NBK > 1

            if diag_b0:
                # pairwise xt/w0 chunks so the (c,m) diagonal consumes in
                # exact arrival order
                for i in range(max(mt, NCH)):
                    if i < mt:
                        load_xt(i)
                    if i < NCH:
                        load_w_chunk(0, i)
            else:
                load_xt(0)
                for c in range(NCH):
                    load_w_chunk(0, c)
                for mi in range(1, mt):
                    load_xt(mi)
            for b in range(1, NBK):
                for c in range(NCH):
                    load_w_chunk(b, c)

            # --- compute ---
            def emit_out(b, m, psum_t, tag="o"):
                c0, width = BLOCKS[b]
                o_sb = o_p.tile([P, width], OUT_DT, tag=tag,
                                name=f"o_{b}_{m}")
                nc.vector.tensor_copy(o_sb[:], psum_t[:])
                nc.scalar.dma_start(
                    out_d[m * P:(m + 1) * P, c0:c0 + width], o_sb[:]
                )

            b_start = 0
            if diag_b0:
                # head block: (chunk, m) diagonal in data-arrival order.
                # 4-MM same-bank bursts let the PE do real work while the
                # rest of the head data streams in.
                b_start = 1
                w0, w0width = BLOCKS[0]
                psums0 = {
                    m: ps.tile([P, w0width], mybir.dt.float32, tag="psum",
                               name=f"psum_0_{m}")
                    for m in range(mt)
                }
                pairs = sorted(
                    ((c, m) for c in range(NCH) for m in range(mt)),
                    key=lambda cm: (cm[0] + cm[1], cm[0]),
                )
                for c, m in pairs:
                    for ko in range(c * WCH, (c + 1) * WCH):
                        w_t, wi = w_c[0][ko]
                        nc.tensor.matmul(
                            psums0[m][:],
                            xt_t[m][:, ko, :],
                            w_t[:, wi, :],
                            start=(ko == 0),
                            stop=(ko == KO - 1),
                        )
                    if c == NCH - 1:
                        emit_out(0, m, psums0[m])

            # remaining blocks: m-major, k-inner (dense same-bank
            # accumulation keeps the PE at the warm back-to-back rate);
            # per-chunk deps let each block's m0 pace with chunk arrival
            for b in range(b_start, NBK):
                c0, width = BLOCKS[b]
                for m in range(mt):
                    last = b == NBK - 1 and m == mt - 1
                    psum_t = ps.tile([P, width], mybir.dt.float32, tag="psum",
                                     name=f"psum_{b}_{m}")
                    for k in range(KO):
                        w_t, wi = w_c[b][k]
                        nc.tensor.matmul(
                            psum_t[:],
                            xt_t[m][:, k, :],
                            w_t[:, wi, :],
                            start=(k == 0),
                            stop=(k == KO - 1),
                        )
                    if last:
                        # split the final block so the tail DMAs are small
                        for h in range(2):
                            hw = width // 2
                            o_sb = o_p.tile([P, hw], OUT_DT,
                                            tag="olast", name=f"o_last{h}")
                            nc.vector.tensor_copy(
                                o_sb[:], psum_t[:, h * hw:(h + 1) * hw])
                            nc.scalar.dma_start(
                                out_d[m * P:(m + 1) * P,
                                      c0 + h * hw:c0 + (h + 1) * hw],
                                o_sb[:],
                            )
                    else:
                        emit_out(b, m, psum_t)

            # close the warm-up accumulation group
            nc.tensor.matmul(wu_ps[:], wu_lhs[:], wu_rhs[:],
                             start=False, stop=True, skip_group_check=True)

    nc.compile()
    return nc


def _swizzle_x(x_pad: np.ndarray, t_pad: int) -> np.ndarray:
    # [t_pad, K] f32 -> [mt, P, KO, P] bf16, xt[mi,p,ko,j] = X[mi*P+j, ko*P+p]
    mt = t_pad // P
    v = x_pad.reshape(mt, P, KO, P).transpose(0, 3, 2, 1)
    return np.ascontiguousarray(v.astype(NP_COMPUTE))


def _swizzle_w(w_g: np.ndarray) -> np.ndarray:
    # [K, N] f32 -> [P, KO, N], w[p,ko,j] = W[ko*P+p, j]
    v = w_g.reshape(KO, P, N).transpose(1, 0, 2)
    return np.ascontiguousarray(v.astype(NP_COMPUTE))


def _run(input, weight, tokens_per_expert, trace=False, **trace_kwargs):
    inp = np.ascontiguousarray(np.asarray(input), dtype=np.float32)
    wgt = np.ascontiguousarray(np.asarray(weight), dtype=np.float32)
    counts = np.asarray(tokens_per_expert).astype(np.int64)
    num_tokens, k = inp.shape
    assert k == K and wgt.shape == (G, K, N)
    # token group boundaries (matches searchsorted(cumsum, arange, 'right')),
    # clamped to the token range for safety on degenerate counts
    ends = np.minimum(np.cumsum(counts), num_tokens)
    starts = np.minimum(ends - counts, num_tokens)
    sizes = np.maximum(ends - starts, 0)

    t_pad = max(P, int(-(-max(int(sizes.max()), 1) // P)) * P)
    nc = _build(t_pad)

    in_maps = []
    for g in range(G):
        x_pad = np.zeros((t_pad, K), dtype=np.float32)
        x_pad[: sizes[g]] = inp[starts[g]:ends[g]]
        in_maps.append({"xt": _swizzle_x(x_pad, t_pad), "w": _swizzle_w(wgt[g])})

    res = bass_utils.run_bass_kernel_spmd(
        nc, in_maps, core_ids=list(range(G)), trace=trace, **trace_kwargs
    )

    # tokens not covered by any expert group get zero output (matches the
    # reference's masked accumulation)
    out = np.zeros((num_tokens, N), dtype=np.float32)
    for g in range(G):
        out[starts[g]:ends[g]] = res.results[g]["out"][: sizes[g]].astype(np.float32)
    return out, res


def kernel(input, weight, tokens_per_expert):
    out, _ = _run(input, weight, tokens_per_expert)
    return out

